# revision 1
# baseline (speedup 1.0000x reference)
"""Trainium2 Bass kernel for chunked flash-attention block (B=2, S=2048, D=1024, H=16).

Sharding: 8 cores = 2 batches x 4 head-groups (4 heads each). Each core computes
its heads' QKV projections + RoPE + per-chunk-softmax attention + its slice of
the output projection; the host sums the 4 partial out-projections per batch.

All device activations are bf16 (fp8 fails the 2e-2 gate: attention output is a
shrinking average, so per-element quantization noise lands full-strength on the
output). The per-head q/k layout puts head_dim on 64-partition blocks
(p = 64*(h%2) + hd) so score matmuls contract over 64 partitions with legal
base partitions {0, 64}.

RoPE pairing is laid out as 32-partition blocks (a-dims in the lower half of
each 64-block, b-dims upper), so the partner swap is four partition-block
copies that run on the otherwise-idle GPSIMD; the cos/sin muls run on DVE with
the sign folded into the per-partition sin table, and GPSIMD does the final
add.

exp() is split between ScalarE (native Exp) and DVE (Schraudolph bit-trick:
i16 = round(x*128/ln2 + 127*128 - C) bitcast to bf16, ~1.8% rms which the
per-chunk softmax ratio mostly tolerates) to keep both engines under the PE
roofline. Units are software-pipelined: scores+exp of unit i+1 are emitted
before W@V+normalize of unit i (drained per-head inside unit i+1's score
loop), and each W@V accumulator is copied raw to SBUF in one op on
alternating engines so its PSUM bank frees immediately; normalize runs from
the SBUF copy off the critical path.

Measured dead ends (don't re-try without a structural change): fp8 anywhere
in the datapath (~1.7% output error per quantized tensor - softmax-averaged
attention output shrinks as fast as noise averages); GPSIMD tensor ops with
a scalar AP or any PSUM operand (walrus rejects at NEFF compile); deeper
score ring / merged small PSUM rings (projection psums live until DVE's
RoPE muls and poison shared rings); any SP DMA reorder other than
wq,wk,ident,x; exp-split deviations from the global 0.56 greedy counter.
"""

import numpy as np
import ml_dtypes

import concourse.bass as bass
import concourse.tile as tile
from concourse import bacc, mybir
from concourse.bass_utils import run_bass_kernel_spmd
from concourse.masks import make_identity

dt = mybir.dt
F32 = dt.float32
BF16 = dt.bfloat16
I16 = dt.int16
AF = mybir.ActivationFunctionType
OP = mybir.AluOpType

B, S, D, H, HD = 2, 2048, 1024, 16, 64
CHUNK = 1024
NHL = 4              # local heads per core
JL = NHL * HD        # 256 local projected dims
LN2 = float(np.log(2.0))
SC_EXP = HD ** -0.5
C_BIT16 = 7.35
BIT_A = SC_EXP * 128.0 / LN2
BIT_B = 127.0 * 128.0 - C_BIT16

# fraction of exp tiles on ScalarE (rest on DVE bit-exp)
ACT_FRAC = 0.56

_CACHED = {}


def _emit_body(nc, tc, persist, rope, aps, rep):
    (x_d, wq_d, wk_d, wv_d, wo_d, c2_d, s2_d, id_d, out_d) = aps
    r = f"r{rep}"

    # ---------------- persistent SBUF tiles + DMA-in --------------------
    x = persist.tile([128, S, 8], BF16, tag="x", name=f"x_{r}")
    x_r = x_d.rearrange("p (s t) -> p s t", t=8)
    wq = persist.tile([128, 8, 256], BF16, tag="wq", name=f"wq_{r}")
    nc.sync.dma_start(wq[:], wq_d.rearrange("p (t g) -> p t g", g=256))
    wk = persist.tile([128, 8, 256], BF16, tag="wk", name=f"wk_{r}")
    nc.sync.dma_start(wk[:], wk_d.rearrange("p (t g) -> p t g", g=256))
    # identity (feeds the PE warm-up + transposes) arrives as the first tiny
    # SP-queue DMA so warm-up starts immediately and GPSIMD stays free
    ident = persist.tile([128, 128], BF16, tag="ident", name=f"ident_{r}")
    nc.sync.dma_start(ident[:], id_d)
    nc.sync.dma_start(x[:, 0:512, :], x_r[:, 0:512, :])
    # cos/sin + late-needed weights go out on the Pool DGE queue so their
    # generation overlaps the SP queue's x/wq/wk stream
    c2h = persist.tile([128, S], BF16, tag="c2h", name=f"c2h_{r}")
    s2h = persist.tile([128, S], BF16, tag="s2h", name=f"s2h_{r}")
    nc.gpsimd.dma_start(c2h[:], c2_d)
    nc.gpsimd.dma_start(s2h[:], s2_d)
    wv = persist.tile([128, 8, 256], BF16, tag="wv", name=f"wv_{r}")
    nc.gpsimd.dma_start(wv[:], wv_d.rearrange("p (t j) -> p t j", j=256))
    wo_sb = persist.tile([128, 2, D], BF16, tag="wo", name=f"wo_{r}")
    nc.gpsimd.dma_start(wo_sb[:], wo_d.rearrange("(t p) n -> p t n", p=128))
    for sb4 in range(1, 4):
        nc.sync.dma_start(x[:, sb4 * 512:(sb4 + 1) * 512, :],
                          x_r[:, sb4 * 512:(sb4 + 1) * 512, :])

    # rotated q/k, bf16, [128 = 2 heads x 64 hd, S]; hd layout per 64-block:
    # lower 32 partitions = even hd (a), upper 32 = odd hd (b)
    qTrA = persist.tile([128, S], BF16, tag="qTrA", name=f"qTrA_{r}")
    qTrB = persist.tile([128, S], BF16, tag="qTrB", name=f"qTrB_{r}")
    kTrA = persist.tile([128, S], BF16, tag="kTrA", name=f"kTrA_{r}")
    kTrB = persist.tile([128, S], BF16, tag="kTrB", name=f"kTrB_{r}")
    qk_tiles = {("q", 0): qTrA, ("q", 1): qTrB, ("k", 0): kTrA, ("k", 1): kTrB}
    # v + ones-column: [128 sk, chunk 2, sk-tile 8, 4h*65]
    vON = persist.tile([128, 2, 8, 260], BF16, tag="vON", name=f"vON_{r}")
    attn = persist.tile([128, 16, JL], BF16, tag="attn", name=f"attn_{r}")

    vON_on = vON[:].rearrange("p c t (h e) -> p c t h e", e=65)
    nc.gpsimd.memset(vON_on[:, :, :, :, 64:65], 1.0)

    with (
        tc.tile_pool(name=f"sc_{r}", bufs=2, space="PSUM") as scp,
        tc.tile_pool(name=f"psb_{r}", bufs=2, space="PSUM") as psbp,
        tc.tile_pool(name=f"pjx_{r}", bufs=2, space="PSUM") as pjp,
        tc.tile_pool(name=f"et_{r}", bufs=9) as etp,
        tc.tile_pool(name=f"nrm_{r}", bufs=4) as nrmp,
    ):
        osbp = nrmp
        atp = nrmp
        # PE warm-up: HAM clock gate keeps a cold PE at reduced rate for the
        # first ~3us; the identity tile is the first (tiny) DMA in.
        warm = scp.tile([128, 2, 512], F32, tag="sc", name=f"warm_{r}")
        for i in range(20):
            nc.tensor.matmul(
                warm[:, i % 2, 0:128],
                lhsT=ident[:, 0:128],
                rhs=ident[:, 0:128],
                start=True, stop=True,
            )
        # prefetch ScalarE's Exp table load (~1.3us) behind the DMA window
        twarm = nrmp.tile([128, 2], F32, tag="rec", name=f"twarm_{r}")
        nc.scalar.activation(out=twarm[:, :], in_=ident[:, 0:2], func=AF.Exp)

        proj_ps = {}

        def emit_qk_projmm(which, st, pair):
            """8 projection matmuls for one (q/k, s-tile, head-pair)."""
            wsb = wq if which == "q" else wk
            sl = slice(st * 512, (st + 1) * 512)
            ps = pjp.tile([128, 512], F32, tag="pj", name=f"pj_{r}")
            proj_ps[(which, st, pair)] = ps
            for kt in range(8):
                nc.tensor.matmul(
                    ps[:],
                    lhsT=wsb[:, kt, pair * 128:(pair + 1) * 128],
                    rhs=x[:, sl, kt],
                    start=(kt == 0), stop=(kt == 7),
                )

        rope_cnt = [0]

        def emit_qk_rope(which, st, pair):
            """RoPE for one projected tile: w2 = ps*sin(+-), t2 = ps*cos (DVE);
            u = 32-block swap of w2 (shifted copies); dst = t2 + u. The first
            few tiles gate the whole pipeline, so they get DVE help with the
            swap instead of riding the slower GPSIMD alone."""
            sl = slice(st * 512, (st + 1) * 512)
            ps = proj_ps.pop((which, st, pair))
            idx = rope_cnt[0]
            rope_cnt[0] += 1
            if idx < 2:
                cp_eng = [nc.vector, nc.gpsimd, nc.vector, nc.gpsimd]
                add_eng = nc.vector
            elif idx < 7:
                cp_eng = [nc.vector, nc.gpsimd, nc.gpsimd, nc.gpsimd]
                add_eng = nc.gpsimd
            else:
                cp_eng = [nc.gpsimd] * 4
                add_eng = nc.gpsimd
            w2 = rope.tile([128, 512], BF16, tag="w2", name=f"w2_{r}")
            nc.vector.tensor_mul(w2[:], ps[:], s2h[:, sl])
            t2 = rope.tile([128, 512], BF16, tag="t2", name=f"t2_{r}")
            nc.vector.tensor_mul(t2[:], ps[:], c2h[:, sl])
            u = rope.tile([128, 512], BF16, tag="u", name=f"u_{r}")
            for blk in range(4):
                o = blk * 32
                so = o ^ 32
                cp_eng[blk].tensor_copy(u[o:o + 32, :], w2[so:so + 32, :])
            dst = qk_tiles[(which, pair)]
            add_eng.tensor_add(dst[:, sl], t2[:], u[:])

        def emit_qk_proj(which, st):
            for pair in range(2):
                emit_qk_projmm(which, st, pair)
                emit_qk_rope(which, st, pair)

        def emit_v_tile(st):
            """One [128 sk, 256 j] v-projection tile -> vON."""
            psv = pjp.tile([128, 512], F32, tag="pj", name=f"pv_{r}")
            for kt in range(8):
                nc.tensor.matmul(
                    psv[:, 0:256],
                    lhsT=x[:, st * 128:(st + 1) * 128, kt],
                    rhs=wv[:, kt, :],
                    start=(kt == 0), stop=(kt == 7),
                )
            nc.scalar.copy(
                vON_on[:, st // 8, st % 8, :, 0:64],
                psv[:, 0:256].rearrange("p (h e) -> p h e", e=64),
            )

        exp_cnt = [0, 0]  # [total, on ACT]
        pending = [None]   # previous unit awaiting W@V, drained per-head

        def emit_exp(sc, et, t2i):
            """exp of one [128, 2, 512] score psum tile into et[:, 2t:2t+2]."""
            exp_cnt[0] += 1
            if exp_cnt[1] < ACT_FRAC * exp_cnt[0]:
                exp_cnt[1] += 1
                nc.scalar.activation(out=et[:, 2 * t2i:2 * t2i + 2, :], in_=sc[:],
                                     func=AF.Exp, scale=SC_EXP)
            else:
                nc.vector.tensor_scalar(et[:, 2 * t2i:2 * t2i + 2, :].bitcast(I16),
                                        sc[:], BIT_A, BIT_B, OP.mult, OP.add)

        def emit_scores_exp(c, n, fillers):
            """Scores+exp for all 4 heads at (chunk c, sq block n).
            fillers: list of up to 16 lists of thunks; list h*4+t2i drains
            right after that head's t2i-th score tile, so PE-side filler work
            is spread between score tiles and the exp engines never see a
            long score gap."""
            ets = []
            for h in range(NHL):
                src = qk_tiles[("q", h // 2)]
                ksrc = qk_tiles[("k", h // 2)]
                hb = 64 * (h % 2)
                et = etp.tile([128, 8, 512], BF16, tag="et", name=f"et_{r}")
                ets.append(et)
                for t2i in range(4):
                    sc = scp.tile([128, 2, 512], F32, tag="sc", name=f"sc_{r}")
                    for par in range(2):
                        tg = c * 8 + t2i * 2 + par
                        nc.tensor.matmul(
                            sc[:, par, :],
                            lhsT=ksrc[hb:hb + 64, tg * 128:(tg + 1) * 128],
                            rhs=src[hb:hb + 64, n * 512:(n + 1) * 512],
                            start=True, stop=True,
                        )
                    slot = h * 4 + t2i
                    if slot < len(fillers):
                        for f in fillers[slot]:
                            f()
                    emit_exp(sc, et, t2i)
                if pending[0] is not None:
                    pc, pn, pets = pending[0]
                    emit_wv_norm_m(pc, pn, pets, h)
            return ets

        def emit_out_m(m):
            """Transpose + output projection + store for one sq tile m."""
            at = atp.tile([128, 2, 128], BF16, tag="at", name=f"at_{r}")
            for jt in range(2):
                tp = pjp.tile([128, 128], BF16, tag="pj", name=f"tp_{r}")
                nc.tensor.transpose(tp[:], attn[:, m, jt * 128:(jt + 1) * 128], ident[:])
                nc.vector.tensor_copy(at[:, jt, :], tp[:])
            osb = osbp.tile([128, 2, 512], BF16, tag="osb", name=f"osb_{r}")
            for nn in range(2):
                pso = pjp.tile([128, 512], F32, tag="pj", name=f"po_{r}")
                for jt in range(2):
                    nc.tensor.matmul(
                        pso[:],
                        lhsT=at[:, jt, :],
                        rhs=wo_sb[:, jt, nn * 512:(nn + 1) * 512],
                        start=(jt == 0), stop=(jt == 1),
                    )
                if nn == 0:
                    nc.scalar.copy(osb[:, nn, :], pso[:])
                else:
                    nc.vector.tensor_copy(osb[:, nn, :], pso[:])
            nc.sync.dma_start(
                out_d[m * 128:(m + 1) * 128, :],
                osb[:].rearrange("p a n -> p (a n)"),
            )

        def emit_wv_norm_m(c, n, ets, m2):
            """W@V + per-chunk-softmax normalize + (chunk 1) output projection
            for one m-tile of block n."""
            if True:
                m = n * 4 + m2
                psB = psbp.tile([128, 264], F32, tag="psb", name=f"psB_{r}")
                psBr = psB[:].rearrange("p (h e) -> p h e", e=66)
                for h in range(NHL):
                    for tg in range(8):
                        nc.tensor.matmul(
                            psB[:, h * 66:h * 66 + 65],
                            lhsT=ets[h][:, tg, m2 * 128:(m2 + 1) * 128],
                            rhs=vON[:, c, tg, h * 65:(h + 1) * 65],
                            start=(h == 0 and tg == 0),
                            stop=(h == NHL - 1 and tg == 7),
                            skip_group_check=True,
                        )
                # copy the raw accumulators to SBUF immediately (frees the
                # psB bank after one op) and normalize from there on GPSIMD,
                # which keeps the exp engines' queues clear of norm work
                braw = nrmp.tile([128, 264], F32, tag="braw", name=f"braw_{r}")
                if m % 2 == 0:
                    nc.scalar.copy(braw[:], psB[:])
                else:
                    nc.vector.tensor_copy(braw[:], psB[:])
                brr = braw[:].rearrange("p (h e) -> p h e", e=66)
                rec = nrmp.tile([128, 4], F32, tag="rec", name=f"rec_{r}")
                nc.vector.reciprocal(rec[:].rearrange("p (h e) -> p h e", e=1),
                                     brr[:, :, 64:65])
                for h in range(NHL):
                    dstp = attn[:, m, h * 64:(h + 1) * 64]
                    if c == 0:
                        nc.scalar.activation(out=dstp, in_=brr[:, h, 0:64],
                                             func=AF.Copy, scale=rec[:, h:h + 1])
                    else:
                        nc.vector.scalar_tensor_tensor(
                            out=dstp, in0=brr[:, h, 0:64], scalar=rec[:, h:h + 1],
                            in1=dstp, op0=OP.mult, op1=OP.add,
                        )
                if c == 1:
                    emit_out_m(m)

        # ---------------- schedule ------------------------------------------
        def pj(which, st, pair):
            return lambda: emit_qk_projmm(which, st, pair)

        def rp(which, st, pair):
            return lambda: emit_qk_rope(which, st, pair)

        def vt(st):
            return lambda: emit_v_tile(st)

        # Only the pair-0 tiles of q0/k0 are rotated before the first unit
        # (heads 0/1); everything else streams through the filler slots one
        # unit ahead of first use.
        emit_qk_projmm("q", 0, 0)
        emit_qk_rope("q", 0, 0)
        emit_qk_projmm("k", 0, 0)
        emit_qk_rope("k", 0, 0)

        # slot layout per unit: [h0s0..h0s3, h1s0.., h2.., h3..]
        # all of vt0-7 must be emitted before the first W@V of unit (0,0),
        # which drains right after head 0 of unit (0,1).
        fill = {
            (0, 0): [[pj("k", 1, 0)], [rp("k", 1, 0)], [pj("q", 0, 1)],
                     [rp("q", 0, 1), pj("k", 0, 1)],
                     [rp("k", 0, 1), pj("k", 1, 1)], [rp("k", 1, 1)],
                     [vt(0)], [vt(1)],
                     [pj("q", 1, 0)], [rp("q", 1, 0)], [pj("q", 1, 1)],
                     [rp("q", 1, 1)], [vt(2)], [vt(3)], [vt(4)], [vt(5)]],
            (0, 1): [[vt(6)], [vt(7)], [pj("q", 2, 0)], [rp("q", 2, 0)],
                     [pj("q", 2, 1)], [rp("q", 2, 1)], [], [], [], [], [], [],
                     [], [], [], []],
            (0, 2): [[pj("q", 3, 0)], [rp("q", 3, 0)], [pj("q", 3, 1)],
                     [rp("q", 3, 1)], [pj("k", 2, 0)], [rp("k", 2, 0)],
                     [pj("k", 2, 1)], [rp("k", 2, 1)], [], [], [], [], [], [], [], []],
            (0, 3): [[pj("k", 3, 0)], [rp("k", 3, 0)], [pj("k", 3, 1)],
                     [rp("k", 3, 1)], [vt(8)], [vt(9)], [vt(10)], [vt(11)],
                     [], [], [], [], [], [], [], []],
            (1, 0): [[vt(12)], [vt(13)], [vt(14)], [vt(15)],
                     [], [], [], [], [], [], [], [], [], [], [], []],
        }
        units = [(c, n) for c in range(2) for n in range(4)]
        for u in units:
            ets = emit_scores_exp(u[0], u[1], fill.get(u, []))
            pending[0] = (u[0], u[1], ets)
        pc, pn, pets = pending[0]
        for m2 in range(4):
            emit_wv_norm_m(pc, pn, pets, m2)


def _build_nc(reps=1):
    nc = bacc.Bacc("TRN2", target_bir_lowering=False, debug=False, num_devices=8)

    aps = (
        nc.dram_tensor("x", [128, S * 8], BF16, kind="ExternalInput").ap(),
        nc.dram_tensor("wq", [128, 8 * JL], BF16, kind="ExternalInput").ap(),
        nc.dram_tensor("wk", [128, 8 * JL], BF16, kind="ExternalInput").ap(),
        nc.dram_tensor("wv", [128, 8 * JL], BF16, kind="ExternalInput").ap(),
        nc.dram_tensor("wo", [JL, D], BF16, kind="ExternalInput").ap(),
        nc.dram_tensor("c2", [128, S], BF16, kind="ExternalInput").ap(),
        nc.dram_tensor("s2", [128, S], BF16, kind="ExternalInput").ap(),
        nc.dram_tensor("ident", [128, 128], BF16, kind="ExternalInput").ap(),
        nc.dram_tensor("out", [S, D], BF16, kind="ExternalOutput").ap(),
    )

    with (
        tile.TileContext(nc) as tc,
        tc.tile_pool(name="persist", bufs=1) as persist,
        tc.tile_pool(name="rope", bufs=4) as rope,
    ):
        for rep in range(reps):
            _emit_body(nc, tc, persist, rope, aps, rep)

    nc.compile()
    return nc


def _get_nc(reps=1):
    if reps not in _CACHED:
        _CACHED[reps] = _build_nc(reps)
    return _CACHED[reps]


def _host_prep(hidden_states, freqs_cis, Wq, Wk, Wv, Wo):
    bf16 = ml_dtypes.bfloat16
    hs = np.asarray(hidden_states, dtype=np.float32)
    fc = np.asarray(freqs_cis, dtype=np.float32)
    Wq = np.asarray(Wq, dtype=np.float32)
    Wk = np.asarray(Wk, dtype=np.float32)
    Wv = np.asarray(Wv, dtype=np.float32)
    Wo = np.asarray(Wo, dtype=np.float32)

    # per-partition cos/sin for hd layout p = 64*hpair + 32*(odd) + f:
    # lower 32 of each 64-block = even hd (freq f = p%32), upper 32 = odd hd.
    # sign: +sin on the a-block (its partner u comes from the b-block and
    # carries -sin), see kernel docstring.
    cos, sin = fc[:, :, 0], fc[:, :, 1]               # [S, 32]
    f_idx = np.arange(128) % 32
    sign = np.where((np.arange(128) % 64) < 32, 1.0, -1.0).astype(np.float32)
    c2 = np.ascontiguousarray(cos.T[f_idx]).astype(bf16)            # [128, S]
    s2 = np.ascontiguousarray(sin.T[f_idx] * sign[:, None]).astype(bf16)

    # dram layouts are per-partition contiguous: x[p, s, t], w[p, t, g]
    xTs = [np.ascontiguousarray(
        hs[b].T.reshape(8, 128, S).transpose(1, 2, 0).reshape(128, S * 8)
    ).astype(bf16) for b in range(B)]

    def packw(w):
        return np.ascontiguousarray(
            w.reshape(8, 128, JL).transpose(1, 0, 2).reshape(128, 8 * JL)
        ).astype(bf16)

    in_maps = []
    for core in range(8):
        b, g = core // 4, core % 4
        jbase = g * JL
        # q/k col perm: per head, evens then odds (a-block, b-block)
        perm = []
        for h in range(NHL):
            perm += [jbase + h * 64 + 2 * f for f in range(32)]
            perm += [jbase + h * 64 + 2 * f + 1 for f in range(32)]
        perm = np.array(perm)
        in_maps.append({
            "x": xTs[b],
            "wq": packw(Wq[:, perm]),
            "wk": packw(Wk[:, perm]),
            "wv": packw(Wv[:, jbase:jbase + JL]),
            "wo": np.ascontiguousarray(Wo[jbase:jbase + JL, :]).astype(bf16),
            "c2": c2,
            "s2": s2,
            "ident": np.eye(128, dtype=np.float32).astype(bf16),
        })
    return in_maps


def kernel(hidden_states, freqs_cis, Wq, Wk, Wv, Wo, _trace=False, _reps=1):
    nc = _get_nc(_reps)
    in_maps = _host_prep(hidden_states, freqs_cis, Wq, Wk, Wv, Wo)
    if _trace:
        try:
            from antenv.axon_hooks import get_axon_ntff_profile_hook  # noqa: F401
        except ImportError:
            _trace = False
    res = run_bass_kernel_spmd(nc, in_maps, core_ids=list(range(8)), trace=_trace)
    outs = [r["out"].astype(np.float32) for r in res.results]
    full = np.zeros((B, S, D), dtype=np.float32)
    for core in range(8):
        full[core // 4] += outs[core]
    if _trace:
        kernel._last_results = res
    return full



# revision 3
# speedup vs baseline: 1.0222x; 1.0222x over previous
"""Trainium2 Bass kernel for chunked flash-attention block (B=2, S=2048, D=1024, H=16).

Sharding: 8 cores = 2 batches x 4 head-groups (4 heads each). Each core computes
its heads' QKV projections + RoPE + per-chunk-softmax attention + its slice of
the output projection; the host sums the 4 partial out-projections per batch.

All device activations are bf16. fp8 anywhere in the datapath fails the 2e-2
gate — numpy-simulated: e4m3 on q/k post-RoPE alone gives 2.1e-2 (score sigma
is 0.41 so exp-argument noise transfers ~1:1 to the output), e4m3 on x/Wq/Wk
adds another 2.8%. Schraudolph bit-exp on the DVE share contributes ~1.2e-2
of the measured 1.31e-2.

The per-head q/k layout puts head_dim on 64-partition blocks (p = 64*(h%2) +
hd) so score matmuls contract over 64 partitions with legal base partitions
{0, 64}. RoPE pairing is laid out as 32-partition blocks (a-dims in the lower
half of each 64-block, b-dims upper), so the partner swap is four partition-
block copies on GPSIMD; cos/sin muls on DVE with the sign folded into the
per-partition sin table.

Schedule: units (c, n) emit 16 score-tile slots in order (h, t2i) =
(0,0),(0,1),(1,0),(1,1),...,(0,2),(0,3),(1,2),... — the t2i<2 slots only
need chunk-first-half k tiles, which pushes the x-quarter-1 DMA deadline ~6
slots later. The PREVIOUS unit's W@V is drained 8 matmuls per slot
(m2 = slot//4, head = slot%4, one tg-group of 8) so the PE paces its score
matmuls to exp throughput instead of blocking on the 2-deep score-psum ring;
the m2 normalize + (chunk-1) out-projection runs on the slot where its 4th
head-group lands. exp() is split ScalarE native / DVE Schraudolph bit-exp by
a global 0.56 greedy counter.

DMA: single SP HWDGE stream ordered by first-use — ident (PE warm-up starts
~2us), wq pair-A, cos/sin first quarter, x q0, wk pair-A, wq/wk pair-B, x q1,
wv, cos/sin rest, x q2, x q3, wo. wq/wk are packed pair-major in DRAM so the
half-weight DMAs are contiguous 2KB/partition runs; cos/sin share one [128,
2, S] tensor so each slice is one DMA. 64 ident warm-up matmuls hold the PE
p-state ramp through the DMA window.

Measured dead ends (don't re-try without a structural change): fp8 anywhere
in the datapath (see above); GPSIMD tensor ops with a scalar AP or any PSUM
operand (walrus rejects at NEFF compile); deeper score ring / merged small
PSUM rings (projection psums live until DVE's RoPE muls and poison shared
rings); exp-split deviations from the global 0.56 greedy counter.
"""

import numpy as np
import ml_dtypes

import concourse.bass as bass
import concourse.tile as tile
from concourse import bacc, mybir
from concourse.bass_utils import run_bass_kernel_spmd
from concourse.masks import make_identity

dt = mybir.dt
F32 = dt.float32
BF16 = dt.bfloat16
I16 = dt.int16
AF = mybir.ActivationFunctionType
OP = mybir.AluOpType

B, S, D, H, HD = 2, 2048, 1024, 16, 64
CHUNK = 1024
NHL = 4              # local heads per core
JL = NHL * HD        # 256 local projected dims
LN2 = float(np.log(2.0))
SC_EXP = HD ** -0.5
C_BIT16 = 7.35
BIT_A = SC_EXP * 128.0 / LN2
BIT_B = 127.0 * 128.0 - C_BIT16

# fraction of exp tiles on ScalarE (rest on DVE bit-exp)
ACT_FRAC = 0.56
WARMUP = 64

_CACHED = {}

# slot order within a unit: all heads' t2i 0/1 first, then t2i 2/3
SLOTS = [(h, t) for h in range(NHL) for t in (0, 1)] + \
        [(h, t) for h in range(NHL) for t in (2, 3)]


def _emit_body(nc, tc, persist, rope, aps, rep):
    (x_d, wq_d, wk_d, wv_d, wo_d, cs_d, id_d, out_d) = aps
    r = f"r{rep}"

    # ---------------- persistent SBUF tiles + DMA-in --------------------
    # single SP HWDGE stream, ordered by first use (see module docstring)
    ident = persist.tile([128, 128], BF16, tag="ident", name=f"ident_{r}")
    nc.sync.dma_start(ident[:], id_d)

    x = persist.tile([128, S, 8], BF16, tag="x", name=f"x_{r}")
    x_r = x_d.rearrange("p (s t) -> p s t", t=8)
    wq = persist.tile([128, 2, 8, 128], BF16, tag="wq", name=f"wq_{r}")
    wq_r = wq_d.rearrange("p (a t g) -> p a t g", a=2, g=128)
    wk = persist.tile([128, 2, 8, 128], BF16, tag="wk", name=f"wk_{r}")
    wk_r = wk_d.rearrange("p (a t g) -> p a t g", a=2, g=128)
    cs = persist.tile([128, 2, S], BF16, tag="cs", name=f"cs_{r}")
    cs_r = cs_d.rearrange("p (a s) -> p a s", a=2)
    wv = persist.tile([128, 8, 256], BF16, tag="wv", name=f"wv_{r}")
    wo_sb = persist.tile([128, 2, D], BF16, tag="wo", name=f"wo_{r}")

    nc.sync.dma_start(wq[:, 0, :, :], wq_r[:, 0, :, :])
    nc.sync.dma_start(cs[:, :, 0:512], cs_r[:, :, 0:512])
    nc.sync.dma_start(x[:, 0:512, :], x_r[:, 0:512, :])
    nc.sync.dma_start(wk[:, 0, :, :], wk_r[:, 0, :, :])
    nc.sync.dma_start(wq[:, 1, :, :], wq_r[:, 1, :, :])
    nc.sync.dma_start(wk[:, 1, :, :], wk_r[:, 1, :, :])
    nc.sync.dma_start(x[:, 512:1024, :], x_r[:, 512:1024, :])
    nc.sync.dma_start(wv[:], wv_d.rearrange("p (t j) -> p t j", j=256))
    nc.sync.dma_start(cs[:, :, 512:S], cs_r[:, :, 512:S])
    nc.sync.dma_start(x[:, 1024:1536, :], x_r[:, 1024:1536, :])
    nc.sync.dma_start(x[:, 1536:2048, :], x_r[:, 1536:2048, :])
    nc.sync.dma_start(wo_sb[:], wo_d.rearrange("(t p) n -> p t n", p=128))

    # rotated q/k, bf16, [128 = 2 heads x 64 hd, S]; hd layout per 64-block:
    # lower 32 partitions = even hd (a), upper 32 = odd hd (b)
    qTrA = persist.tile([128, S], BF16, tag="qTrA", name=f"qTrA_{r}")
    qTrB = persist.tile([128, S], BF16, tag="qTrB", name=f"qTrB_{r}")
    kTrA = persist.tile([128, S], BF16, tag="kTrA", name=f"kTrA_{r}")
    kTrB = persist.tile([128, S], BF16, tag="kTrB", name=f"kTrB_{r}")
    qk_tiles = {("q", 0): qTrA, ("q", 1): qTrB, ("k", 0): kTrA, ("k", 1): kTrB}
    # v + ones-column: [128 sk, chunk 2, sk-tile 8, 4h*65]
    vON = persist.tile([128, 2, 8, 260], BF16, tag="vON", name=f"vON_{r}")
    attn = persist.tile([128, 16, JL], BF16, tag="attn", name=f"attn_{r}")

    vON_on = vON[:].rearrange("p c t (h e) -> p c t h e", e=65)
    nc.gpsimd.memset(vON_on[:, :, :, :, 64:65], 1.0)

    with (
        tc.tile_pool(name=f"sc_{r}", bufs=2, space="PSUM") as scp,
        tc.tile_pool(name=f"psb_{r}", bufs=2, space="PSUM") as psbp,
        tc.tile_pool(name=f"pjx_{r}", bufs=2, space="PSUM") as pjp,
        tc.tile_pool(name=f"et_{r}", bufs=9) as etp,
        tc.tile_pool(name=f"nrm_{r}", bufs=4) as nrmp,
    ):
        osbp = nrmp
        atp = nrmp
        # PE warm-up: HAM clock gate keeps a cold PE at reduced rate for the
        # first ~3us; the identity tile is the first (tiny) DMA in, and the
        # warm-up must span the whole input-DMA window or the ramp resets.
        warm = scp.tile([128, 2, 512], F32, tag="sc", name=f"warm_{r}")
        for i in range(WARMUP):
            nc.tensor.matmul(
                warm[:, i % 2, 0:128],
                lhsT=ident[:, 0:128],
                rhs=ident[:, 0:128],
                start=True, stop=True,
            )
        # prefetch ScalarE's Exp table load (~1.3us) behind the DMA window
        twarm = nrmp.tile([128, 2], F32, tag="rec", name=f"twarm_{r}")
        nc.scalar.activation(out=twarm[:, :], in_=ident[:, 0:2], func=AF.Exp)

        proj_ps = {}

        def emit_qk_projmm(which, st, pair):
            """8 projection matmuls for one (q/k, s-tile, head-pair)."""
            wsb = wq if which == "q" else wk
            sl = slice(st * 512, (st + 1) * 512)
            ps = pjp.tile([128, 512], F32, tag="pj", name=f"pj_{r}")
            proj_ps[(which, st, pair)] = ps
            for kt in range(8):
                nc.tensor.matmul(
                    ps[:],
                    lhsT=wsb[:, pair, kt, :],
                    rhs=x[:, sl, kt],
                    start=(kt == 0), stop=(kt == 7),
                )

        rope_cnt = [0]

        def emit_qk_rope(which, st, pair):
            """RoPE for one projected tile: w2 = ps*sin(+-), t2 = ps*cos (DVE);
            u = 32-block swap of w2 (shifted copies); dst = t2 + u. The first
            few tiles gate the whole pipeline, so they get DVE help with the
            swap instead of riding the slower GPSIMD alone."""
            sl = slice(st * 512, (st + 1) * 512)
            ps = proj_ps.pop((which, st, pair))
            idx = rope_cnt[0]
            rope_cnt[0] += 1
            if idx < 2:
                cp_eng = [nc.vector, nc.gpsimd, nc.vector, nc.gpsimd]
                add_eng = nc.vector
            elif idx < 7:
                cp_eng = [nc.vector, nc.gpsimd, nc.gpsimd, nc.gpsimd]
                add_eng = nc.gpsimd
            else:
                cp_eng = [nc.gpsimd] * 4
                add_eng = nc.gpsimd
            w2 = rope.tile([128, 512], BF16, tag="w2", name=f"w2_{r}")
            nc.vector.tensor_mul(w2[:], ps[:], cs[:, 1, sl])
            t2 = rope.tile([128, 512], BF16, tag="t2", name=f"t2_{r}")
            nc.vector.tensor_mul(t2[:], ps[:], cs[:, 0, sl])
            u = rope.tile([128, 512], BF16, tag="u", name=f"u_{r}")
            for blk in range(4):
                o = blk * 32
                so = o ^ 32
                cp_eng[blk].tensor_copy(u[o:o + 32, :], w2[so:so + 32, :])
            dst = qk_tiles[(which, pair)]
            add_eng.tensor_add(dst[:, sl], t2[:], u[:])

        def emit_qk_proj(which, st):
            for pair in range(2):
                emit_qk_projmm(which, st, pair)
                emit_qk_rope(which, st, pair)

        def emit_v_tile(st):
            """One [128 sk, 256 j] v-projection tile -> vON."""
            psv = pjp.tile([128, 512], F32, tag="pj", name=f"pv_{r}")
            for kt in range(8):
                nc.tensor.matmul(
                    psv[:, 0:256],
                    lhsT=x[:, st * 128:(st + 1) * 128, kt],
                    rhs=wv[:, kt, :],
                    start=(kt == 0), stop=(kt == 7),
                )
            nc.scalar.copy(
                vON_on[:, st // 8, st % 8, :, 0:64],
                psv[:, 0:256].rearrange("p (h e) -> p h e", e=64),
            )

        exp_cnt = [0, 0]  # [total, on ACT]
        pending = [None]   # previous unit awaiting W@V, drained per-slot

        def emit_exp(sc, et, t2i):
            """exp of one [128, 2, 512] score psum tile into et[:, 2t:2t+2]."""
            exp_cnt[0] += 1
            if exp_cnt[1] < ACT_FRAC * exp_cnt[0]:
                exp_cnt[1] += 1
                nc.scalar.activation(out=et[:, 2 * t2i:2 * t2i + 2, :], in_=sc[:],
                                     func=AF.Exp, scale=SC_EXP)
            else:
                nc.vector.tensor_scalar(et[:, 2 * t2i:2 * t2i + 2, :].bitcast(I16),
                                        sc[:], BIT_A, BIT_B, OP.mult, OP.add)

        def emit_out_m(m):
            """Transpose + output projection + store for one sq tile m."""
            at = atp.tile([128, 2, 128], BF16, tag="at", name=f"at_{r}")
            for jt in range(2):
                tp = pjp.tile([128, 128], BF16, tag="pj", name=f"tp_{r}")
                nc.tensor.transpose(tp[:], attn[:, m, jt * 128:(jt + 1) * 128], ident[:])
                nc.vector.tensor_copy(at[:, jt, :], tp[:])
            osb = osbp.tile([128, 2, 512], BF16, tag="osb", name=f"osb_{r}")
            for nn in range(2):
                pso = pjp.tile([128, 512], F32, tag="pj", name=f"po_{r}")
                for jt in range(2):
                    nc.tensor.matmul(
                        pso[:],
                        lhsT=at[:, jt, :],
                        rhs=wo_sb[:, jt, nn * 512:(nn + 1) * 512],
                        start=(jt == 0), stop=(jt == 1),
                    )
                if nn == 0:
                    nc.scalar.copy(osb[:, nn, :], pso[:])
                else:
                    nc.vector.tensor_copy(osb[:, nn, :], pso[:])
            nc.sync.dma_start(
                out_d[m * 128:(m + 1) * 128, :],
                osb[:].rearrange("p a n -> p (a n)"),
            )

        wv_ps = {}

        def emit_wv_sub(c, n, ets, m2, hh):
            """One tg-group (8 matmuls) of the W@V accumulation for m-tile m2,
            head hh, of the pending unit."""
            if hh == 0:
                wv_ps[m2] = psbp.tile([128, 264], F32, tag="psb",
                                      name=f"psB_{r}")
            psB = wv_ps[m2]
            for tg in range(8):
                nc.tensor.matmul(
                    psB[:, hh * 66:hh * 66 + 65],
                    lhsT=ets[hh][:, tg, m2 * 128:(m2 + 1) * 128],
                    rhs=vON[:, c, tg, hh * 65:(hh + 1) * 65],
                    start=(hh == 0 and tg == 0),
                    stop=(hh == NHL - 1 and tg == 7),
                    skip_group_check=True,
                )

        def emit_wv_finish(c, n, m2):
            """per-chunk-softmax normalize + (chunk 1) output projection for
            one m-tile whose W@V accumulation just completed."""
            m = n * 4 + m2
            psB = wv_ps.pop(m2)
            # copy the raw accumulators to SBUF immediately (frees the psB
            # bank after one op) and normalize from there, keeping the exp
            # engines' queues clear of norm work
            braw = nrmp.tile([128, 264], F32, tag="braw", name=f"braw_{r}")
            if m % 2 == 0:
                nc.scalar.copy(braw[:], psB[:])
            else:
                nc.vector.tensor_copy(braw[:], psB[:])
            brr = braw[:].rearrange("p (h e) -> p h e", e=66)
            rec = nrmp.tile([128, 4], F32, tag="rec", name=f"rec_{r}")
            nc.vector.reciprocal(rec[:].rearrange("p (h e) -> p h e", e=1),
                                 brr[:, :, 64:65])
            for h in range(NHL):
                dstp = attn[:, m, h * 64:(h + 1) * 64]
                if c == 0:
                    nc.scalar.activation(out=dstp, in_=brr[:, h, 0:64],
                                         func=AF.Copy, scale=rec[:, h:h + 1])
                else:
                    nc.vector.scalar_tensor_tensor(
                        out=dstp, in0=brr[:, h, 0:64], scalar=rec[:, h:h + 1],
                        in1=dstp, op0=OP.mult, op1=OP.add,
                    )
            if c == 1:
                emit_out_m(m)

        def emit_drain_slot(s):
            """Pending-unit W@V work for slot s: one 8-matmul tg-group."""
            if pending[0] is None:
                return
            pc, pn, pets = pending[0]
            emit_wv_sub(pc, pn, pets, s // 4, s % 4)
            if s % 4 == 3:
                emit_wv_finish(pc, pn, s // 4)

        def emit_unit(c, n, fillers):
            """Scores+exp for all 16 slots of unit (c, n), with the pending
            unit's W@V interleaved one tg-group per slot."""
            ets = {}
            for s, (h, t2i) in enumerate(SLOTS):
                if h not in ets:
                    ets[h] = etp.tile([128, 8, 512], BF16, tag="et",
                                      name=f"et_{r}")
                src = qk_tiles[("q", h // 2)]
                ksrc = qk_tiles[("k", h // 2)]
                hb = 64 * (h % 2)
                sc = scp.tile([128, 2, 512], F32, tag="sc", name=f"sc_{r}")
                for par in range(2):
                    tg = c * 8 + t2i * 2 + par
                    nc.tensor.matmul(
                        sc[:, par, :],
                        lhsT=ksrc[hb:hb + 64, tg * 128:(tg + 1) * 128],
                        rhs=src[hb:hb + 64, n * 512:(n + 1) * 512],
                        start=True, stop=True,
                    )
                if s < len(fillers):
                    for f in fillers[s]:
                        f()
                emit_exp(sc, ets[h], t2i)
                emit_drain_slot(s)
            return [ets[h] for h in range(NHL)]

        # ---------------- schedule ------------------------------------------
        def pj(which, st, pair):
            return lambda: emit_qk_projmm(which, st, pair)

        def rp(which, st, pair):
            return lambda: emit_qk_rope(which, st, pair)

        def vt(st):
            return lambda: emit_v_tile(st)

        # Only the pair-0 tiles of q0/k0 are rotated before the first unit
        # (heads 0/1); everything else streams through the filler slots.
        # Deadlines: pair-B tiles by slot 2 (head 2's first score), k st1 by
        # slot 8 (t2i=2), vON tg0-7 complete before unit (0,1) slot 0's
        # drain (vt6/vt7 lead that unit's slots, emitted before the drain).
        emit_qk_projmm("q", 0, 0)
        emit_qk_rope("q", 0, 0)
        emit_qk_projmm("k", 0, 0)
        emit_qk_rope("k", 0, 0)

        fill = {
            (0, 0): [[pj("q", 0, 1)], [rp("q", 0, 1)], [pj("k", 0, 1)],
                     [rp("k", 0, 1)], [pj("k", 1, 0)], [rp("k", 1, 0)],
                     [pj("k", 1, 1)], [rp("k", 1, 1)], [pj("q", 1, 0)],
                     [rp("q", 1, 0)], [pj("q", 1, 1)], [rp("q", 1, 1)],
                     [vt(0), vt(1)], [vt(2), vt(3)], [vt(4), vt(5)],
                     [vt(6), vt(7)]],
            (0, 1): [[pj("q", 2, 0)], [rp("q", 2, 0)],
                     [pj("q", 2, 1)], [rp("q", 2, 1)], [], [], [], [], [], [],
                     [], [], [], [], [], []],
            (0, 2): [[pj("q", 3, 0)], [rp("q", 3, 0)], [pj("q", 3, 1)],
                     [rp("q", 3, 1)], [pj("k", 2, 0)], [rp("k", 2, 0)],
                     [pj("k", 2, 1)], [rp("k", 2, 1)], [], [], [], [], [], [], [], []],
            (0, 3): [[pj("k", 3, 0)], [rp("k", 3, 0)], [pj("k", 3, 1)],
                     [rp("k", 3, 1)], [vt(8)], [vt(9)], [vt(10)], [vt(11)],
                     [], [], [], [], [], [], [], []],
            (1, 0): [[vt(12)], [vt(13)], [vt(14)], [vt(15)],
                     [], [], [], [], [], [], [], [], [], [], [], []],
        }
        units = [(c, n) for c in range(2) for n in range(4)]
        for u in units:
            ets = emit_unit(u[0], u[1], fill.get(u, []))
            pending[0] = (u[0], u[1], ets)
        pc, pn, pets = pending[0]
        for s in range(16):
            emit_wv_sub(pc, pn, pets, s // 4, s % 4)
            if s % 4 == 3:
                emit_wv_finish(pc, pn, s // 4)


def _build_nc(reps=1):
    nc = bacc.Bacc("TRN2", target_bir_lowering=False, debug=False, num_devices=8)

    aps = (
        nc.dram_tensor("x", [128, S * 8], BF16, kind="ExternalInput").ap(),
        nc.dram_tensor("wq", [128, 2 * 8 * 128], BF16, kind="ExternalInput").ap(),
        nc.dram_tensor("wk", [128, 2 * 8 * 128], BF16, kind="ExternalInput").ap(),
        nc.dram_tensor("wv", [128, 8 * JL], BF16, kind="ExternalInput").ap(),
        nc.dram_tensor("wo", [JL, D], BF16, kind="ExternalInput").ap(),
        nc.dram_tensor("cs", [128, 2 * S], BF16, kind="ExternalInput").ap(),
        nc.dram_tensor("ident", [128, 128], BF16, kind="ExternalInput").ap(),
        nc.dram_tensor("out", [S, D], BF16, kind="ExternalOutput").ap(),
    )

    with (
        tile.TileContext(nc) as tc,
        tc.tile_pool(name="persist", bufs=1) as persist,
        tc.tile_pool(name="rope", bufs=4) as rope,
    ):
        for rep in range(reps):
            _emit_body(nc, tc, persist, rope, aps, rep)

    nc.compile()
    return nc


def _get_nc(reps=1):
    if reps not in _CACHED:
        _CACHED[reps] = _build_nc(reps)
    return _CACHED[reps]


def _host_prep(hidden_states, freqs_cis, Wq, Wk, Wv, Wo):
    bf16 = ml_dtypes.bfloat16
    hs = np.asarray(hidden_states, dtype=np.float32)
    fc = np.asarray(freqs_cis, dtype=np.float32)
    Wq = np.asarray(Wq, dtype=np.float32)
    Wk = np.asarray(Wk, dtype=np.float32)
    Wv = np.asarray(Wv, dtype=np.float32)
    Wo = np.asarray(Wo, dtype=np.float32)

    # per-partition cos/sin for hd layout p = 64*hpair + 32*(odd) + f:
    # lower 32 of each 64-block = even hd (a), upper 32 = odd hd (b).
    # sign: +sin on the a-block (its partner u comes from the b-block and
    # carries -sin). cs row 0 = cos, row 1 = sin, interleaved per partition.
    cos, sin = fc[:, :, 0], fc[:, :, 1]               # [S, 32]
    f_idx = np.arange(128) % 32
    sign = np.where((np.arange(128) % 64) < 32, 1.0, -1.0).astype(np.float32)
    c2 = np.ascontiguousarray(cos.T[f_idx]).astype(np.float32)      # [128, S]
    s2 = np.ascontiguousarray(sin.T[f_idx] * sign[:, None]).astype(np.float32)
    cs2 = np.stack([c2, s2], axis=1).reshape(128, 2 * S).astype(bf16)

    # dram layouts are per-partition contiguous: x[p, s, t], w[p, pair, t, g]
    xTs = [np.ascontiguousarray(
        hs[b].T.reshape(8, 128, S).transpose(1, 2, 0).reshape(128, S * 8)
    ).astype(bf16) for b in range(B)]

    def packw_pair(w):
        # [D, 256] -> [128 p, 2 pair, 8 t, 128 g]
        return np.ascontiguousarray(
            w.reshape(8, 128, 2, 128).transpose(1, 2, 0, 3).reshape(128, 2 * 8 * 128)
        ).astype(bf16)

    def packw(w):
        return np.ascontiguousarray(
            w.reshape(8, 128, JL).transpose(1, 0, 2).reshape(128, 8 * JL)
        ).astype(bf16)

    in_maps = []
    for core in range(8):
        b, g = core // 4, core % 4
        jbase = g * JL
        # q/k col perm: per head, evens then odds (a-block, b-block)
        perm = []
        for h in range(NHL):
            perm += [jbase + h * 64 + 2 * f for f in range(32)]
            perm += [jbase + h * 64 + 2 * f + 1 for f in range(32)]
        perm = np.array(perm)
        in_maps.append({
            "x": xTs[b],
            "wq": packw_pair(Wq[:, perm]),
            "wk": packw_pair(Wk[:, perm]),
            "wv": packw(Wv[:, jbase:jbase + JL]),
            "wo": np.ascontiguousarray(Wo[jbase:jbase + JL, :]).astype(bf16),
            "cs": cs2,
            "ident": np.eye(128, dtype=np.float32).astype(bf16),
        })
    return in_maps


def kernel(hidden_states, freqs_cis, Wq, Wk, Wv, Wo, _trace=False, _reps=1):
    nc = _get_nc(_reps)
    in_maps = _host_prep(hidden_states, freqs_cis, Wq, Wk, Wv, Wo)
    if _trace:
        try:
            from antenv.axon_hooks import get_axon_ntff_profile_hook  # noqa: F401
        except ImportError:
            _trace = False
    res = run_bass_kernel_spmd(nc, in_maps, core_ids=list(range(8)), trace=_trace)
    outs = [r["out"].astype(np.float32) for r in res.results]
    full = np.zeros((B, S, D), dtype=np.float32)
    for core in range(8):
        full[core // 4] += outs[core]
    if _trace:
        kernel._last_results = res
    return full


# revision 56
# speedup vs baseline: 1.0435x; 1.0209x over previous
"""Trainium2 Bass kernel for chunked flash-attention block (B=2, S=2048, D=1024, H=16).

Sharding: 8 cores = 2 batches x 4 head-groups (4 heads each). Each core computes
its heads' QKV projections + RoPE + per-chunk-softmax attention + its slice of
the output projection; the host sums the 4 partial out-projections per batch.

All device activations are bf16. fp8 anywhere in the datapath fails the 2e-2
gate — numpy-simulated: e4m3 on q/k post-RoPE alone gives 2.1e-2 (score sigma
is 0.41 so exp-argument noise transfers ~1:1 to the output), e4m3 on x/Wq/Wk
adds another 2.8%. Schraudolph bit-exp on the DVE share contributes ~1.2e-2
of the measured 1.31e-2.

The per-head q/k layout puts head_dim on 64-partition blocks (p = 64*(h%2) +
hd) so score matmuls contract over 64 partitions with legal base partitions
{0, 64}. RoPE pairing is laid out as 32-partition blocks (a-dims in the lower
half of each 64-block, b-dims upper), so the partner swap is four partition-
block copies on GPSIMD; cos/sin muls on DVE with the sign folded into the
per-partition sin table.

Schedule: units (c, n) emit 16 score-tile slots in order (h, t2i) =
(0,0),(0,1),(1,0),(1,1),...,(0,2),(0,3),(1,2),... — the t2i<2 slots only
need chunk-first-half k tiles, which pushes the x-quarter-1 DMA deadline ~6
slots later. The PREVIOUS unit's W@V is drained 8 matmuls per slot
(m2 = slot//4, head = slot%4, one tg-group of 8) so the PE paces its score
matmuls to exp throughput instead of blocking on the 2-deep score-psum ring;
the m2 normalize + (chunk-1) out-projection runs on the slot where its 4th
head-group lands. exp() is split ScalarE native / DVE Schraudolph bit-exp by
a global 0.56 greedy counter.

DMA: single SP HWDGE stream ordered by first-use — ident (PE warm-up starts
~2us), wq pair-A, cos/sin first quarter, x q0, wk pair-A, wq/wk pair-B, x q1,
wv, cos/sin rest, x q2, x q3, wo. wq/wk are packed pair-major in DRAM so the
half-weight DMAs are contiguous 2KB/partition runs; cos/sin share one [128,
2, S] tensor so each slice is one DMA. 64 ident warm-up matmuls hold the PE
p-state ramp through the DMA window.

Measured dead ends (don't re-try without a structural change): fp8 anywhere
in the datapath (see above); GPSIMD tensor ops with a scalar AP or any PSUM
operand (walrus rejects at NEFF compile); deeper score ring / merged small
PSUM rings (projection psums live until DVE's RoPE muls and poison shared
rings); exp-split deviations from the global 0.56 greedy counter.
"""

import numpy as np
import ml_dtypes

import concourse.bass as bass
import concourse.tile as tile
from concourse import bacc, mybir
from concourse.bass_utils import run_bass_kernel_spmd
from concourse.masks import make_identity

dt = mybir.dt
F32 = dt.float32
BF16 = dt.bfloat16
I16 = dt.int16
AF = mybir.ActivationFunctionType
OP = mybir.AluOpType

B, S, D, H, HD = 2, 2048, 1024, 16, 64
CHUNK = 1024
NHL = 4              # local heads per core
JL = NHL * HD        # 256 local projected dims
LN2 = float(np.log(2.0))
SC_EXP = HD ** -0.5
C_BIT16 = 7.35
BIT_A = SC_EXP * 128.0 / LN2
BIT_B = 127.0 * 128.0 - C_BIT16

WARMUP = 76

# emission-time cost estimates (ns) for the adaptive ACT/DVE balancer,
# derived from the TimelineSim cost model (elems*cycle + access-init + seq)
def _act_cost(elems, psum=True):
    return elems * 0.833 + 217
def _dve_cost(elems, psum=True, fast=1.0):
    return elems * 1.0417 * fast + (170 if psum else 105)

_CACHED = {}

# slot order within a unit: all heads' t2i 0/1 first, then t2i 2/3
SLOTS = [(h, t) for h in range(NHL) for t in (0, 1)] + \
        [(h, t) for h in range(NHL) for t in (2, 3)]


def _emit_body(nc, tc, persist, rope, aps, rep):
    (x_d, wq_d, wk_d, wv_d, wo_d, cs_d, id_d, out_d) = aps
    r = f"r{rep}"

    # ---------------- persistent SBUF tiles + DMA-in --------------------
    # single SP HWDGE stream, ordered by first use (see module docstring)
    ident = persist.tile([128, 128], BF16, tag="ident", name=f"ident_{r}")
    nc.sync.dma_start(ident[:], id_d)

    x = persist.tile([128, S, 8], BF16, tag="x", name=f"x_{r}")
    x_r = x_d.rearrange("p (s t) -> p s t", t=8)
    wq = persist.tile([128, 2, 8, 128], BF16, tag="wq", name=f"wq_{r}")
    wq_r = wq_d.rearrange("p (a t g) -> p a t g", a=2, g=128)
    wk = persist.tile([128, 2, 8, 128], BF16, tag="wk", name=f"wk_{r}")
    wk_r = wk_d.rearrange("p (a t g) -> p a t g", a=2, g=128)
    cs = persist.tile([128, 2, S], BF16, tag="cs", name=f"cs_{r}")
    cs_r = cs_d.rearrange("p (a s) -> p a s", a=2)
    wv = persist.tile([128, 8, 256], BF16, tag="wv", name=f"wv_{r}")
    wo_sb = persist.tile([128, 2, D], BF16, tag="wo", name=f"wo_{r}")

    nc.sync.dma_start(wq[:, 0, :, :], wq_r[:, 0, :, :])
    nc.sync.dma_start(cs[:, :, 0:512], cs_r[:, :, 0:512])
    nc.sync.dma_start(x[:, 0:512, :], x_r[:, 0:512, :])
    nc.sync.dma_start(wk[:, 0, :, :], wk_r[:, 0, :, :])
    nc.sync.dma_start(wv[:], wv_d.rearrange("p (t j) -> p t j", j=256))
    nc.sync.dma_start(wq[:, 1, :, :], wq_r[:, 1, :, :])
    nc.sync.dma_start(wk[:, 1, :, :], wk_r[:, 1, :, :])
    nc.sync.dma_start(x[:, 512:1024, :], x_r[:, 512:1024, :])
    nc.sync.dma_start(cs[:, :, 512:S], cs_r[:, :, 512:S])
    nc.sync.dma_start(x[:, 1024:1536, :], x_r[:, 1024:1536, :])
    nc.sync.dma_start(x[:, 1536:2048, :], x_r[:, 1536:2048, :])
    nc.sync.dma_start(wo_sb[:], wo_d.rearrange("(t p) n -> p t n", p=128))

    # rotated q/k, bf16, [128 = 2 heads x 64 hd, S]; hd layout per 64-block:
    # lower 32 partitions = even hd (a), upper 32 = odd hd (b)
    qTrA = persist.tile([128, S], BF16, tag="qTrA", name=f"qTrA_{r}")
    qTrB = persist.tile([128, S], BF16, tag="qTrB", name=f"qTrB_{r}")
    kTrA = persist.tile([128, S], BF16, tag="kTrA", name=f"kTrA_{r}")
    kTrB = persist.tile([128, S], BF16, tag="kTrB", name=f"kTrB_{r}")
    qk_tiles = {("q", 0): qTrA, ("q", 1): qTrB, ("k", 0): kTrA, ("k", 1): kTrB}
    # v + ones-column: [128 sk, chunk 2, sk-tile 8, 4h*65]
    vON = persist.tile([128, 2, 8, 260], BF16, tag="vON", name=f"vON_{r}")
    attn = persist.tile([128, 16, JL], BF16, tag="attn", name=f"attn_{r}")

    vON_on = vON[:].rearrange("p c t (h e) -> p c t h e", e=65)
    nc.gpsimd.memset(vON_on[:, :, :, :, 64:65], 1.0)

    with (
        tc.tile_pool(name=f"sc_{r}", bufs=2, space="PSUM") as scp,
        tc.tile_pool(name=f"psb_{r}", bufs=2, space="PSUM") as psbp,
        tc.tile_pool(name=f"pjx_{r}", bufs=2, space="PSUM") as pjp,
        tc.tile_pool(name=f"et_{r}", bufs=9) as etp,
        tc.tile_pool(name=f"nrm_{r}", bufs=6) as nrmp,
    ):
        osbp = nrmp
        atp = nrmp
        # PE warm-up: HAM clock gate keeps a cold PE at reduced rate for the
        # first ~3us; the identity tile is the first (tiny) DMA in, and the
        # warm-up must span the whole input-DMA window or the ramp resets.
        warm = scp.tile([128, 2, 512], F32, tag="sc", name=f"warm_{r}")
        for i in range(WARMUP):
            nc.tensor.matmul(
                warm[:, i % 2, 0:128],
                lhsT=ident[:, 0:128],
                rhs=ident[:, 0:128],
                start=True, stop=True,
            )
        # prefetch ScalarE's Exp table load (~1.3us) behind the DMA window
        twarm = nrmp.tile([128, 2], F32, tag="rec", name=f"twarm_{r}")
        nc.scalar.activation(out=twarm[:, :], in_=ident[:, 0:2], func=AF.Exp)

        proj_ps = {}

        def emit_qk_projmm(which, st, pair, half=None):
            """8 projection matmuls for one (q/k, s-tile, head-pair); with
            half=0/1 only kt 0-3 / 4-7 are emitted (the psum accumulation
            group stays open between the two half-thunks)."""
            wsb = wq if which == "q" else wk
            sl = slice(st * 512, (st + 1) * 512)
            if half in (None, 0):
                ps = pjp.tile([128, 512], F32, tag="pj", name=f"pj_{r}")
                proj_ps[(which, st, pair)] = ps
            else:
                ps = proj_ps[(which, st, pair)]
            kts = range(8) if half is None else range(half * 4, half * 4 + 4)
            for kt in kts:
                nc.tensor.matmul(
                    ps[:],
                    lhsT=wsb[:, pair, kt, :],
                    rhs=x[:, sl, kt],
                    start=(kt == 0), stop=(kt == 7),
                    skip_group_check=True,
                )

        def emit_qk_rope(which, st, pair):
            """RoPE for one projected tile: w2 = ps*sin(+-), t2 = ps*cos
            (DVE, psum-bound); then the partner swap and add are FUSED as four
            partition-shifted adds dst[o:o+32] = t2[o:o+32] + w2[o^32:...]
            (sign folded in the sin table). All-SBUF bf16, so DVE runs them
            in its 4x packed mode (~283ns) vs GPSIMD's 0.42-efficiency add
            (~1111ns); the balancer trickles some to the otherwise-idle Pool."""
            sl = slice(st * 512, (st + 1) * 512)
            ps = proj_ps.pop((which, st, pair))
            w2 = rope.tile([128, 512], BF16, tag="w2", name=f"w2_{r}")
            load["dve"] += 2 * _dve_cost(512)
            nc.vector.tensor_mul(w2[:], ps[:], cs[:, 1, sl])
            t2 = rope.tile([128, 512], BF16, tag="t2", name=f"t2_{r}")
            nc.vector.tensor_mul(t2[:], ps[:], cs[:, 0, sl])
            u = rope.tile([128, 512], BF16, tag="u", name=f"u_{r}")
            dst = qk_tiles[(which, pair)]
            # walrus requires same start partition on all TensorTensor APs,
            # so the partner swap stays as shifted copies (legal), balanced
            # between DVE's 4x packed mode and the otherwise-idle Pool
            for blk in range(4):
                o = blk * 32
                so = o ^ 32
                if load["dve"] + 283 <= load["pool"] + 806:
                    load["dve"] += 283
                    eng = nc.vector
                else:
                    load["pool"] += 806
                    eng = nc.gpsimd
                eng.tensor_copy(u[o:o + 32, :], w2[so:so + 32, :])
            if load["dve"] + 283 <= load["pool"] + 1111:
                load["dve"] += 283
                eng = nc.vector
            else:
                load["pool"] += 1111
                eng = nc.gpsimd
            eng.tensor_add(dst[:, sl], t2[:], u[:])

        def emit_qk_proj(which, st):
            for pair in range(2):
                emit_qk_projmm(which, st, pair)
                emit_qk_rope(which, st, pair)

        def emit_v_tile(st):
            """One [128 sk, 256 j] v-projection tile -> vON."""
            psv = pjp.tile([128, 512], F32, tag="pj", name=f"pv_{r}")
            for kt in range(8):
                nc.tensor.matmul(
                    psv[:, 0:256],
                    lhsT=x[:, st * 128:(st + 1) * 128, kt],
                    rhs=wv[:, kt, :],
                    start=(kt == 0), stop=(kt == 7),
                )
            load["act"] += _act_cost(256)
            nc.scalar.copy(
                vON_on[:, st // 8, st % 8, :, 0:64],
                psv[:, 0:256].rearrange("p (h e) -> p h e", e=64),
            )

        pending = [None]   # previous unit awaiting W@V, drained per-slot

        # adaptive engine balancer: cumulative ns of work assigned to each
        # engine; choosers put the next op on the engine with the smaller
        # projected backlog. Fixed-engine ops (RoPE muls, stt, act-scale)
        # tally too so the exp split auto-shifts per phase.
        load = {"act": 0.0, "dve": 0.0, "pool": 0.0}

        def pick(act_ns, dve_ns):
            if load["act"] + act_ns <= load["dve"] + dve_ns:
                load["act"] += act_ns
                return "act"
            load["dve"] += dve_ns
            return "dve"

        def emit_exp(sc, et, t2i, force=None):
            """exp of one [128, 2, 512] score psum tile into et[:, 2t:2t+2]."""
            if force == "act":
                load["act"] += _act_cost(1024)
                eng = "act"
            else:
                eng = pick(_act_cost(1024), _dve_cost(1024))
            if eng == "act":
                nc.scalar.activation(out=et[:, 2 * t2i:2 * t2i + 2, :], in_=sc[:],
                                     func=AF.Exp, scale=SC_EXP)
            else:
                nc.vector.tensor_scalar(et[:, 2 * t2i:2 * t2i + 2, :].bitcast(I16),
                                        sc[:], BIT_A, BIT_B, OP.mult, OP.add)

        def copy_pick(dst, src, elems, fast=1.0):
            """PSUM-evacuation copy on whichever engine is less loaded."""
            if pick(_act_cost(elems), _dve_cost(elems, fast=fast)) == "act":
                nc.scalar.copy(dst, src)
            else:
                nc.vector.tensor_copy(dst, src)

        def emit_out_m(m):
            """Transpose + output projection + store for one sq tile m."""
            at = atp.tile([128, 2, 128], BF16, tag="at", name=f"at_{r}")
            tp = pjp.tile([128, 2, 128], BF16, tag="pj", name=f"tp_{r}")
            for jt in range(2):
                nc.tensor.transpose(tp[:, jt, :], attn[:, m, jt * 128:(jt + 1) * 128],
                                    ident[:])
            copy_pick(at[:], tp[:], 256, fast=0.5)
            osb = osbp.tile([128, 2, 512], BF16, tag="osb", name=f"osb_{r}")
            out_r = out_d[m * 128:(m + 1) * 128, :].rearrange(
                "p (a n) -> p a n", a=2)
            for nn in range(2):
                pso = pjp.tile([128, 512], F32, tag="pj", name=f"po_{r}")
                for jt in range(2):
                    nc.tensor.matmul(
                        pso[:],
                        lhsT=at[:, jt, :],
                        rhs=wo_sb[:, jt, nn * 512:(nn + 1) * 512],
                        start=(jt == 0), stop=(jt == 1),
                    )
                copy_pick(osb[:, nn, :], pso[:], 512)
                # per-half DMA: the first half's descriptor generation and
                # transfer overlap the second half's matmul + copy, shortening
                # the end-of-program latency chain
                nc.sync.dma_start(out_r[:, nn, :], osb[:, nn, :])

        wv_ps = {}

        def emit_wv_sub(c, n, ets, m2, hh):
            """One tg-group (8 matmuls) of the W@V accumulation for m-tile m2,
            head hh, of the pending unit."""
            if hh == 0:
                wv_ps[m2] = psbp.tile([128, 264], F32, tag="psb",
                                      name=f"psB_{r}")
            psB = wv_ps[m2]
            for tg in range(8):
                nc.tensor.matmul(
                    psB[:, hh * 66:hh * 66 + 65],
                    lhsT=ets[hh][:, tg, m2 * 128:(m2 + 1) * 128],
                    rhs=vON[:, c, tg, hh * 65:(hh + 1) * 65],
                    start=(hh == 0 and tg == 0),
                    stop=(hh == NHL - 1 and tg == 7),
                    skip_group_check=True,
                )

        def emit_wv_finish(c, n, m2):
            """per-chunk-softmax normalize + (chunk 1) output projection for
            one m-tile whose W@V accumulation just completed."""
            m = n * 4 + m2
            psB = wv_ps.pop(m2)
            # copy the raw accumulators to SBUF immediately (frees the psB
            # bank after one op) and normalize from there, keeping the exp
            # engines' queues clear of norm work. bf16 is enough precision
            # (numerator and denominator round together) and enables the DVE
            # 2x/4x packed modes on the normalize ops.
            braw = nrmp.tile([128, 264], BF16, tag="braw", name=f"braw_{r}")
            copy_pick(braw[:], psB[:], 264)
            brr = braw[:].rearrange("p (h e) -> p h e", e=66)
            rec = nrmp.tile([128, 4], F32, tag="rec", name=f"rec_{r}")
            load["dve"] += 110
            nc.vector.reciprocal(rec[:].rearrange("p (h e) -> p h e", e=1),
                                 brr[:, :, 64:65])
            for h in range(NHL):
                dstp = attn[:, m, h * 64:(h + 1) * 64]
                if c == 0:
                    load["act"] += _act_cost(64, psum=False)
                    nc.scalar.activation(out=dstp, in_=brr[:, h, 0:64],
                                         func=AF.Copy, scale=rec[:, h:h + 1])
                else:
                    load["dve"] += _dve_cost(64, psum=False, fast=0.5)
                    nc.vector.scalar_tensor_tensor(
                        out=dstp, in0=brr[:, h, 0:64], scalar=rec[:, h:h + 1],
                        in1=dstp, op0=OP.mult, op1=OP.add,
                    )
            if c == 1:
                out_queue.append(m)

        out_queue = []

        def emit_drain_slot(s):
            """Pending-unit W@V work for slot s: one 8-matmul tg-group, plus
            deferred out-projections. Outs go preferentially to the slots of
            units draining a chunk-0 pending (those have the least engine and
            PE load); units draining chunk-1 take at most one per 4 slots."""
            pop_ok = False
            if pending[0] is not None:
                pc, pn, pets = pending[0]
                emit_wv_sub(pc, pn, pets, s // 4, s % 4)
                if s % 4 == 3:
                    emit_wv_finish(pc, pn, s // 4)
                pop_ok = (s % 2 == 1) if pc == 0 else (s % 4 == 1)
            if out_queue and pop_ok:
                emit_out_m(out_queue.pop(0))

        def emit_unit(c, n, fillers, slots=SLOTS, selfdrain=False):
            """Scores+exp for all 16 slots of unit (c, n), with the pending
            unit's W@V interleaved one tg-group per slot. exp is emitted
            before the fillers so the exp engines' queues are never blocked
            behind RoPE/norm work when the score ring is tight."""
            ets = {}
            for s, (h, t2i) in enumerate(slots):
                if h not in ets:
                    ets[h] = etp.tile([128, 8, 512], BF16, tag="et",
                                      name=f"et_{r}")
                src = qk_tiles[("q", h // 2)]
                ksrc = qk_tiles[("k", h // 2)]
                hb = 64 * (h % 2)
                sc = scp.tile([128, 2, 512], F32, tag="sc", name=f"sc_{r}")
                for par in range(2):
                    tg = c * 8 + t2i * 2 + par
                    nc.tensor.matmul(
                        sc[:, par, :],
                        lhsT=ksrc[hb:hb + 64, tg * 128:(tg + 1) * 128],
                        rhs=src[hb:hb + 64, n * 512:(n + 1) * 512],
                        start=True, stop=True,
                    )
                emit_exp(sc, ets[h], t2i)
                if s < len(fillers):
                    for f in fillers[s]:
                        f()
                emit_drain_slot(s)
                if selfdrain and s >= 12:
                    # last unit, head-major slots: its own ets[0..2] are done
                    # by s12/s13/s14 and ets[3] right at s15, so the first
                    # m-tile's W@V can start here instead of in the tail
                    emit_wv_sub(c, n, [ets[hh] for hh in range(NHL)], 0, s - 12)
            return [ets[h] for h in range(NHL)]

        # ---------------- schedule ------------------------------------------
        def pj(which, st, pair):
            return lambda: emit_qk_projmm(which, st, pair)

        def pjh(which, st, pair, half):
            return lambda: emit_qk_projmm(which, st, pair, half)

        def rp(which, st, pair):
            return lambda: emit_qk_rope(which, st, pair)

        def vt(st):
            return lambda: emit_v_tile(st)

        # Only the pair-0 tiles of q0/k0 are rotated before the first unit
        # (heads 0/1); everything else streams through the filler slots.
        # Deadlines: pair-B tiles by slot 2 (head 2's first score), k st1 by
        # slot 8 (t2i=2), vON tg0-7 complete before unit (0,1) slot 0's
        # drain (vt6/vt7 lead that unit's slots, emitted before the drain).
        emit_qk_projmm("q", 0, 0)
        emit_qk_rope("q", 0, 0)
        emit_qk_projmm("k", 0, 0)
        emit_qk_rope("k", 0, 0)
        # vt0/vt1 fill the PE while the k00A rope chain drains (wv lands
        # right after wk pair-A in the DMA stream)
        emit_v_tile(0)
        emit_v_tile(1)

        fill = {
            (0, 0): [[pj("q", 0, 1)], [rp("q", 0, 1)], [pj("k", 0, 1)],
                     [rp("k", 0, 1)], [pj("k", 1, 0)], [rp("k", 1, 0)],
                     [pj("k", 1, 1)], [rp("k", 1, 1)], [pj("q", 1, 0)],
                     [rp("q", 1, 0)], [pj("q", 1, 1)], [rp("q", 1, 1)],
                     [vt(2)], [vt(3)], [vt(4), vt(5)],
                     [vt(6), vt(7)]],
            (0, 1): [[pjh("q", 2, 0, 0)], [pjh("q", 2, 0, 1)], [],
                     [rp("q", 2, 0)], [], [], [pjh("q", 2, 1, 0)],
                     [pjh("q", 2, 1, 1)], [], [rp("q", 2, 1)],
                     [], [], [], [], [], []],
            (0, 2): [[pjh("q", 3, 0, 0)], [pjh("q", 3, 0, 1)],
                     [rp("q", 3, 0)], [], [pjh("q", 3, 1, 0)],
                     [pjh("q", 3, 1, 1)], [rp("q", 3, 1)], [],
                     [pjh("k", 2, 0, 0)], [pjh("k", 2, 0, 1)],
                     [rp("k", 2, 0)], [], [pjh("k", 2, 1, 0)],
                     [pjh("k", 2, 1, 1)], [rp("k", 2, 1)], []],
            (0, 3): [[pjh("k", 3, 0, 0)], [pjh("k", 3, 0, 1)],
                     [rp("k", 3, 0)], [], [pjh("k", 3, 1, 0)],
                     [pjh("k", 3, 1, 1)], [rp("k", 3, 1)], [],
                     [vt(8)], [], [vt(9)], [], [vt(10)], [], [vt(11)], []],
            (1, 0): [[vt(12)], [], [], [], [vt(13)], [], [], [],
                     [vt(14)], [], [], [], [vt(15)], [], [], []],
        }
        units = [(c, n) for c in range(2) for n in range(4)]
        slots_last = [(h, t) for h in range(NHL) for t in range(4)]
        for u in units:
            last = u == units[-1]
            ets = emit_unit(u[0], u[1], fill.get(u, []),
                            slots=slots_last if last else SLOTS,
                            selfdrain=last)
            pending[0] = (u[0], u[1], ets)
        pc, pn, pets = pending[0]
        emit_wv_finish(pc, pn, 0)   # m2=0 accumulated in the last unit's slots
        for s in range(4, 16):
            emit_wv_sub(pc, pn, pets, s // 4, s % 4)
            if s % 4 == 3:
                emit_wv_finish(pc, pn, s // 4)
            if out_queue:
                emit_out_m(out_queue.pop(0))
        while out_queue:
            emit_out_m(out_queue.pop(0))


def _build_nc(reps=1):
    nc = bacc.Bacc("TRN2", target_bir_lowering=False, debug=False, num_devices=8)

    aps = (
        nc.dram_tensor("x", [128, S * 8], BF16, kind="ExternalInput").ap(),
        nc.dram_tensor("wq", [128, 2 * 8 * 128], BF16, kind="ExternalInput").ap(),
        nc.dram_tensor("wk", [128, 2 * 8 * 128], BF16, kind="ExternalInput").ap(),
        nc.dram_tensor("wv", [128, 8 * JL], BF16, kind="ExternalInput").ap(),
        nc.dram_tensor("wo", [JL, D], BF16, kind="ExternalInput").ap(),
        nc.dram_tensor("cs", [128, 2 * S], BF16, kind="ExternalInput").ap(),
        nc.dram_tensor("ident", [128, 128], BF16, kind="ExternalInput").ap(),
        nc.dram_tensor("out", [S, D], BF16, kind="ExternalOutput").ap(),
    )

    with (
        tile.TileContext(nc) as tc,
        tc.tile_pool(name="persist", bufs=1) as persist,
        tc.tile_pool(name="rope", bufs=6) as rope,
    ):
        for rep in range(reps):
            _emit_body(nc, tc, persist, rope, aps, rep)

    nc.compile()
    return nc


def _get_nc(reps=1):
    if reps not in _CACHED:
        _CACHED[reps] = _build_nc(reps)
    return _CACHED[reps]


def _host_prep(hidden_states, freqs_cis, Wq, Wk, Wv, Wo):
    bf16 = ml_dtypes.bfloat16
    hs = np.asarray(hidden_states, dtype=np.float32)
    fc = np.asarray(freqs_cis, dtype=np.float32)
    Wq = np.asarray(Wq, dtype=np.float32)
    Wk = np.asarray(Wk, dtype=np.float32)
    Wv = np.asarray(Wv, dtype=np.float32)
    Wo = np.asarray(Wo, dtype=np.float32)

    # per-partition cos/sin for hd layout p = 64*hpair + 32*(odd) + f:
    # lower 32 of each 64-block = even hd (a), upper 32 = odd hd (b).
    # sign: +sin on the a-block (its partner u comes from the b-block and
    # carries -sin). cs row 0 = cos, row 1 = sin, interleaved per partition.
    cos, sin = fc[:, :, 0], fc[:, :, 1]               # [S, 32]
    f_idx = np.arange(128) % 32
    sign = np.where((np.arange(128) % 64) < 32, 1.0, -1.0).astype(np.float32)
    c2 = np.ascontiguousarray(cos.T[f_idx]).astype(np.float32)      # [128, S]
    s2 = np.ascontiguousarray(sin.T[f_idx] * sign[:, None]).astype(np.float32)
    cs2 = np.stack([c2, s2], axis=1).reshape(128, 2 * S).astype(bf16)

    # dram layouts are per-partition contiguous: x[p, s, t], w[p, pair, t, g]
    xTs = [np.ascontiguousarray(
        hs[b].T.reshape(8, 128, S).transpose(1, 2, 0).reshape(128, S * 8)
    ).astype(bf16) for b in range(B)]

    def packw_pair(w):
        # [D, 256] -> [128 p, 2 pair, 8 t, 128 g]
        return np.ascontiguousarray(
            w.reshape(8, 128, 2, 128).transpose(1, 2, 0, 3).reshape(128, 2 * 8 * 128)
        ).astype(bf16)

    def packw(w):
        return np.ascontiguousarray(
            w.reshape(8, 128, JL).transpose(1, 0, 2).reshape(128, 8 * JL)
        ).astype(bf16)

    in_maps = []
    for core in range(8):
        b, g = core // 4, core % 4
        jbase = g * JL
        # q/k col perm: per head, evens then odds (a-block, b-block)
        perm = []
        for h in range(NHL):
            perm += [jbase + h * 64 + 2 * f for f in range(32)]
            perm += [jbase + h * 64 + 2 * f + 1 for f in range(32)]
        perm = np.array(perm)
        in_maps.append({
            "x": xTs[b],
            "wq": packw_pair(Wq[:, perm]),
            "wk": packw_pair(Wk[:, perm]),
            "wv": packw(Wv[:, jbase:jbase + JL]),
            "wo": np.ascontiguousarray(Wo[jbase:jbase + JL, :]).astype(bf16),
            "cs": cs2,
            "ident": np.eye(128, dtype=np.float32).astype(bf16),
        })
    return in_maps


def kernel(hidden_states, freqs_cis, Wq, Wk, Wv, Wo, _trace=False, _reps=1):
    nc = _get_nc(_reps)
    in_maps = _host_prep(hidden_states, freqs_cis, Wq, Wk, Wv, Wo)
    if _trace:
        try:
            from antenv.axon_hooks import get_axon_ntff_profile_hook  # noqa: F401
        except ImportError:
            _trace = False
    res = run_bass_kernel_spmd(nc, in_maps, core_ids=list(range(8)), trace=_trace)
    outs = [r["out"].astype(np.float32) for r in res.results]
    full = np.zeros((B, S, D), dtype=np.float32)
    for core in range(8):
        full[core // 4] += outs[core]
    if _trace:
        kernel._last_results = res
    return full
